# revision 44
# baseline (speedup 1.0000x reference)
"""Trainium2 Bass kernel for an ODE-RNN encoder (z0 posterior).

Model: 128-step reversed-time GRU-like recurrence with an Euler ODE step on
the mean channel, then a final transform producing (mean_z0, std_z0).

Strategy: data-parallel over the subject (batch) dim across 8 NeuronCores,
weights replicated.  Everything runs on-chip in a transposed layout
([feature, batch], batch=256 on the free dim).  v2 rework versus the first
working kernel (1.79 ms) — the step was a single serialized dependency
chain; every engine change targets either the chain or the busiest engine
(PE array at 67%):
- PE: -8 matmuls/step.  neg_eye and ns2-bias matmuls fold into DVE
  scalar_tensor_tensor ops; the observation-mask broadcast matmul is
  replaced by a host-precomputed DMA'd [128,B] mask tile; the ns1 net
  multiplies the reset gate directly into the state (am = r.yode,
  as = r.ys) instead of the 0.5-prescale base+correction split (the base
  matmuls bought PE utilization when the PE wasn't the bottleneck).
- ACT: 15 -> 11 ops.  IDENTITY and ABS move to DVE; softplus(x) =
  log1p(e^x) via one Newton step needs only TWO exps using
  (1+e^x)*e^{-y0} == (1+u)*2^{-u} with u = e^{-|x|}, which holds exactly
  for both signs of x (y0 = relu(x) + ln2*u).
- sigmoid(z) = 0.5 + 0.5*tanh(z/2) keeps every transcendental in the
  resident `exp_and_others` ACT table set (no per-step table switches);
- chain: m-half wavefront on the r-gate and ns stages (tanh per 128-row
  half so layer-2 matmuls start after the first half), DVE chain ops
  grouped back-to-back (no cross-engine sem between them), ys-tail in
  P/Q form so only two DVE ops trail the last exp;
- slack-tolerant softplus ops (rl, s1, sA, P2, P) run on the otherwise
  idle GPSIMD/Pool engine;
- matmul operands and the recurrent state are bf16 (fp32 PSUM
  accumulate): fp32 matmuls lower to TWO half-speed PE passes, bf16 is
  single-pass with fast weight load.
"""
import sys
import numpy as np
import ml_dtypes

for _p in ("/opt/trn_rl_repo", "/root/.axon_site/_ro/trn_rl_repo"):
    if _p not in sys.path:
        sys.path.append(_p)

N_SUBJ, N_TP, INPUT_DIM, LATENT, N_UNIT = 2048, 128, 64, 128, 256
HALF = INPUT_DIM // 2
N_CORES = 8
B = N_SUBJ // N_CORES          # 256 subjects per core (free dim)
L = LATENT
LN2 = float(np.log(2.0))
BF = ml_dtypes.bfloat16

_CACHE = {}


# --------------------------------------------------------------------------
# Bass program
# --------------------------------------------------------------------------
def _build(n_tp, dts):
    import concourse.mybir as mybir
    from concourse import bacc, tile

    F32 = mybir.dt.float32
    B16 = mybir.dt.bfloat16
    AF = mybir.ActivationFunctionType
    OP = mybir.AluOpType

    # Bacc (not plain Bass): its compile() legalizes the TRN2 one-sync-wait-
    # per-instruction limit (event-semaphore splitting, matmul-wait moves).
    nc = bacc.Bacc(None)

    # ---- DRAM I/O ----
    d_x = nc.dram_tensor("x_rev", [n_tp, INPUT_DIM, B], B16, kind="ExternalInput")
    d_m = nc.dram_tensor("m_bc", [n_tp, 128, B], B16, kind="ExternalInput")

    bspec = {  # bf16 weights (matmul operands)
        "ug1_k0": [L, N_UNIT], "ug1_k1": [L, N_UNIT], "ug1_kx": [INPUT_DIM + 1, N_UNIT],
        "rg1_k0": [L, N_UNIT], "rg1_k1": [L, N_UNIT], "rg1_kx": [INPUT_DIM + 1, N_UNIT],
        "ns1_k0": [L, N_UNIT], "ns1_k1": [L, N_UNIT], "ns1_kx": [INPUT_DIM + 1, N_UNIT],
        "ode1_w": [L, N_UNIT], "ode_b2r": [1, 128],
        "ode1_b1r0": [1, 128], "ode1_b1r1": [1, 128],
        "neg_eye": [L, L],
        "ode2_k0": [128, L], "ode2_k1": [128, L],
        "ug2_k0": [128, L], "ug2_k1": [128, L],
        "rg2_k0": [128, L], "rg2_k1": [128, L],
        "ns2_k0": [128, 2 * L], "ns2_k1": [128, 2 * L],
        "tz1_k0": [L, N_UNIT], "tz1_k1": [L, N_UNIT], "tz1_b": [1, N_UNIT],
        "tz2_k0": [128, 2 * L], "tz2_k1": [128, 2 * L],
    }
    fspec = {  # fp32 per-partition columns (ACT bias / DVE scalar APs)
        "ug2_bc": [128, 1], "rg2_bc": [128, 1],
        "ns2_bm": [128, 1], "ns2_bs": [128, 1], "tz2_bm": [128, 1], "tz2_bs": [128, 1],
    }
    d_w = {k: nc.dram_tensor(k, v, B16, kind="ExternalInput") for k, v in bspec.items()}
    d_w.update({k: nc.dram_tensor(k, v, F32, kind="ExternalInput")
                for k, v in fspec.items()})

    d_om = nc.dram_tensor("out_m", [L, B], F32, kind="ExternalOutput")
    d_os = nc.dram_tensor("out_s", [L, B], F32, kind="ExternalOutput")

    CC = float(np.float32(1e-6) - np.float32(1.0))

    with tile.TileContext(nc) as tc:
        with (
            tc.tile_pool(name="const", bufs=1) as cp,
            tc.tile_pool(name="work", bufs=3) as wp,
            tc.tile_pool(name="ps", bufs=1, space="PSUM") as pp,
        ):
            # ---- resident constants / weights ----
            w = {}
            for k, shp in bspec.items():
                w[k] = cp.tile(shp, B16, tag=k, name=k)
                nc.sync.dma_start(w[k][:], d_w[k][:])
            for k, shp in fspec.items():
                w[k] = cp.tile(shp, F32, tag=k, name=k)
                nc.sync.dma_start(w[k][:], d_w[k][:])
            ones_row = cp.tile([1, B], B16, tag="ones_row", name="ones_row")
            nc.vector.memset(ones_row[:], 1.0)

            xbufs = []
            for j in range(3):
                xb = cp.tile([INPUT_DIM + 1, B], B16, tag=f"xb{j}", name=f"xb{j}")
                nc.vector.memset(xb[INPUT_DIM:, :], 1.0)
                xbufs.append(xb)
            mbufs = [cp.tile([128, B], B16, tag=f"mb{j}", name=f"mb{j}")
                     for j in range(3)]

            # state lives in bf16 (the bf16 matmul-input rounding dominates;
            # bf16 state adds nothing measurable)
            ym = [cp.tile([L, B], B16, tag=f"ym{i}", name=f"ym{i}") for i in range(2)]
            ys = [cp.tile([L, B], B16, tag=f"ys{i}", name=f"ys{i}") for i in range(2)]
            nc.vector.memset(ym[0][:], 0.0)
            nc.vector.memset(ys[0][:], 0.0)

            mm = nc.tensor.matmul

            # Warm the PE's clock past every weight DMA with K=1 dummy
            # matmuls so steady-state matmuls only wait on one producer.
            scr = pp.tile([1, 16], F32, tag="scr", name="scr")
            for k in bspec:
                mm(scr[0:1, 0:1], w[k][0:1, 0:1], w[k][0:1, 1:2],
                   start=True, stop=True)
            # DVE/ACT read fp32 DMA-produced columns: warm those clocks too
            nf = len(fspec)
            warm_dv = cp.tile([1, 2 * nf], F32, tag="warm_dv", name="warm_dv")
            for j, k in enumerate(fspec):
                nc.vector.tensor_copy(warm_dv[0:1, j:j + 1], w[k][0:1, 0:1])
                nc.scalar.copy(warm_dv[0:1, nf + j:nf + j + 1], w[k][0:1, 0:1])

            # ---- the recurrence ----
            # psB (ode1 hidden pre-act) is accumulated via the distributed
            # form ode1@Ym' == ode1@Yode + ode1@pm (+ b1 K=1 rows), so step
            # t+1's psB starts filling mid-step-t and only the ode1@pm part
            # sits on the recurrence cycle.  Emission order == scheduler
            # priority: within each engine, ops are emitted in intended
            # steady-state execution order.
            from concourse.tile_rust import add_dep_helper
            prev_rg2k1 = None
            prev_tail = None
            psB = pp.tile([128, 2 * B], F32, tag="psB", name="psB_init")
            for m in range(2):
                mm(psB[:, m * B:(m + 1) * B], w[f"ode1_b1r{m}"][:],
                   ones_row[:], start=(m == 0), stop=(m == 1))
            for t in range(n_tp):
                cur, nxt = t % 2, (t + 1) % 2
                last = t == n_tp - 1
                dt_t = float(dts[t])
                xb = xbufs[t % 3]
                mb = mbufs[t % 3]
                nc.sync.dma_start(xb[:INPUT_DIM, :], d_x[t])
                nc.sync.dma_start(mb[:], d_m[t])
                # absorb the x-DMA wait into a K=1 dummy
                mm(scr[0:1, 0:1], xb[0:1, 0:1], xb[0:1, 1:2], start=True, stop=True)

                # PSUM banks (one start=True per bank per step)
                psAr = pp.tile([128, 2 * B], F32, tag="psAr", name="psAr")
                psAu = pp.tile([128, 2 * B], F32, tag="psAu", name="psAu")
                psC = pp.tile([128, 2 * B], F32, tag="psC", name="psC")
                psD = pp.tile([128, 2 * B], F32, tag="psD", name="psD")
                psE = pp.tile([128, 2 * B], F32, tag="psE", name="psE")
                psF = pp.tile([128, B], F32, tag="psF", name="psF")

                # ODE hidden of THIS step: psB was fully accumulated during
                # step t-1; single merged tanh (b1 already in psB)
                h_ode = wp.tile([128, 2 * B], B16, tag="h_ode", name="h_ode")
                nc.scalar.activation(h_ode[:], psB[:], AF.Tanh)

                # psF: b2 row first (ready instantly, opens the bank), then
                # ode2 halves as h_ode lands
                i_b2 = mm(psF[:], w["ode_b2r"][:], ones_row[:],
                          start=True, stop=False)
                # x-parts of the u/r gates: ready early (banks freed by the
                # previous step's h_r/h_u tanhs), keep the PE fed -- but not
                # at the cost of delaying the previous step's rg2 tail
                for net, ps in (("rg1", psAr), ("ug1", psAu)):
                    for m in range(2):
                        ms = slice(m * 128, (m + 1) * 128)
                        i_kx = mm(ps[:, m * B:(m + 1) * B], w[net + "_kx"][:, ms],
                                  xb[:], start=(m == 0), stop=False)
                        if net == "rg1" and m == 0 and prev_rg2k1 is not None:
                            add_dep_helper(i_kx.ins, prev_rg2k1.ins, False,
                                           "pe-order")
                for m in range(2):
                    mm(psF[:], w[f"ode2_k{m}"][:], h_ode[:, m * B:(m + 1) * B],
                       start=False, stop=(m == 1))

                # Yode = Ym + dt*(ode_out + b2): ONE DVE op
                yode = wp.tile([L, B], B16, tag="yode", name="yode")
                nc.vector.scalar_tensor_tensor(yode[:], psF[:], dt_t, ym[cur][:],
                                               op0=OP.mult, op1=OP.add)
                # deferred ys' of the previous step: emitted HERE so it sits
                # after yode in the DVE static order (yode must not queue
                # behind the whole previous tail)
                A2 = None
                if prev_tail is not None:
                    pPpre, pT1, pQ = prev_tail
                    A2 = wp.tile([L, B], B16, tag="A2", name="A2")
                    i_A2 = nc.vector.tensor_tensor(A2[:], pPpre[:], pT1[:],
                                                   op=OP.add)
                    # Q (ys-cycle) must not queue behind A2 on DVE
                    add_dep_helper(i_A2.ins, prev_iQ.ins, False, "dve-order")
                    nc.vector.tensor_tensor(ys[cur][:], A2[:], pQ[:], op=OP.add)
                    pQt = pQ
                    prev_tail = None

                # gate layer 1 state parts: yode (k0) BEFORE ys (k1) in PE
                # order -- yode is ready first in steady state
                for net, ps in (("rg1", psAr), ("ug1", psAu)):
                    for m in range(2):
                        ms = slice(m * 128, (m + 1) * 128)
                        mm(ps[:, m * B:(m + 1) * B], w[net + "_k0"][:, ms], yode[:],
                           start=False, stop=False)
                # next step's psB state part: ode1 @ Yode + b1 rows
                if not last:
                    psBn = pp.tile([128, 2 * B], F32, tag="psB", name=f"psBn{t}")
                    i_y0 = None
                    for m in range(2):
                        ms = slice(m * 128, (m + 1) * 128)
                        i_y = mm(psBn[:, m * B:(m + 1) * B], w["ode1_w"][:, ms],
                                 yode[:], start=(m == 0), stop=False)
                        if m == 0:
                            i_y0 = i_y
                    for m in range(2):
                        i_b1 = mm(psBn[:, m * B:(m + 1) * B],
                                  w[f"ode1_b1r{m}"][:], ones_row[:],
                                  start=False, stop=False)
                        # bank-start order: the b1 rows are ready from t=0
                        # and must not be scheduled before the start=True mm
                        add_dep_helper(i_b1.ins, i_y0.ins, False, "bank-start")
                # r-gate ys part DISTRIBUTED: k1@ys' == k1@A2 + k1@Q, and Q
                # (then A2) land well before ys' itself -- the r path starts
                # ~1us earlier.  The u-gate keeps the plain ys form (slack).
                if A2 is not None:
                    for src in (pQt, A2):
                        for m in range(2):
                            ms = slice(m * 128, (m + 1) * 128)
                            mm(psAr[:, m * B:(m + 1) * B], w["rg1_k1"][:, ms],
                               src[:], start=False, stop=(src is A2 and m == 1))
                else:
                    for m in range(2):
                        ms = slice(m * 128, (m + 1) * 128)
                        mm(psAr[:, m * B:(m + 1) * B], w["rg1_k1"][:, ms],
                           ys[cur][:], start=False, stop=(m == 1))
                for m in range(2):
                    ms = slice(m * 128, (m + 1) * 128)
                    mm(psAu[:, m * B:(m + 1) * B], w["ug1_k1"][:, ms], ys[cur][:],
                       start=False, stop=(m == 1))

                # r-gate layer 2 with m-half wavefront; h_u after h_r-m1
                h_r = wp.tile([128, 2 * B], B16, tag="h_r", name="h_r")
                t_ur = wp.tile([128, 2 * B], B16, tag="t_ur", name="t_ur")
                i_hr1 = i_rg2k1 = None
                for m in range(2):
                    i_hr1 = nc.scalar.activation(h_r[:, m * B:(m + 1) * B],
                                                 psAr[:, m * B:(m + 1) * B], AF.Tanh)
                    i_rg2k1 = mm(psD[:, B:], w[f"rg2_k{m}"][:],
                                 h_r[:, m * B:(m + 1) * B],
                                 start=(m == 0), stop=(m == 1))
                nc.scalar.activation(t_ur[:, B:], psD[:, B:], AF.Tanh,
                                     bias=w["rg2_bc"][:, 0:1], scale=0.5)
                h_u = wp.tile([128, 2 * B], B16, tag="h_u", name="h_u")
                i_hu = nc.scalar.activation(h_u[:], psAu[:], AF.Tanh)
                # ACT order: the off-cycle merged h_u must not run between
                # the two h_r halves (the r path is the critical cycle)
                add_dep_helper(i_hu.ins, i_hr1.ins, False, "act-order")
                for m in range(2):
                    mm(psD[:, 0:B], w[f"ug2_k{m}"][:], h_u[:, m * B:(m + 1) * B],
                       start=False, stop=(m == 1))
                nc.scalar.activation(t_ur[:, 0:B], psD[:, 0:B], AF.Tanh,
                                     bias=w["ug2_bc"][:, 0:1], scale=0.5)

                # reset products via the prescale trick: ns1_k0/k1 carry a
                # host-side 0.5 factor, so r.state == 0.5*(1+tanh)*state
                # becomes ONE stt per channel: (tanh + 1) * state
                am = wp.tile([L, B], B16, tag="am", name="am")
                nc.vector.scalar_tensor_tensor(am[:], t_ur[:, B:], 1.0, yode[:],
                                               op0=OP.add, op1=OP.mult)
                a_s = wp.tile([L, B], B16, tag="a_s", name="a_s")
                nc.vector.scalar_tensor_tensor(a_s[:], t_ur[:, B:], 1.0, ys[cur][:],
                                               op0=OP.add, op1=OP.mult)
                # G = m * (0.5 - 0.5*t_u)   (== m * (1 - sigmoid(zU)))
                q_u = wp.tile([L, B], B16, tag="q_u", name="q_u")
                nc.vector.tensor_scalar(q_u[:], t_ur[:, 0:B], -0.5, 0.5,
                                        op0=OP.mult, op1=OP.add)
                g = wp.tile([L, B], B16, tag="g", name="g")
                nc.vector.tensor_tensor(g[:], q_u[:], mb[:L, :], op=OP.mult)
                gl = wp.tile([L, B], B16, tag="gl", name="gl")
                nc.vector.tensor_scalar(gl[:], g[:], LN2, None, op0=OP.mult)

                # ns1: x-part (psC WAR on previous h_ns clears mid-prev-step)
                # then the r-gated state parts, am before as per m-half
                for m in range(2):
                    ms = slice(m * 128, (m + 1) * 128)
                    mm(psC[:, m * B:(m + 1) * B], w["ns1_kx"][:, ms], xb[:],
                       start=(m == 0), stop=False)
                for m in range(2):
                    sl = psC[:, m * B:(m + 1) * B]
                    ms = slice(m * 128, (m + 1) * 128)
                    mm(sl, w["ns1_k0"][:, ms], am[:], start=False, stop=False)
                    mm(sl, w["ns1_k1"][:, ms], a_s[:], start=False, stop=(m == 1))

                # ns layer 2 with m-half wavefront; NM first (mean cycle),
                # neg_eye (-Yode, ready early) fills the h_ns-m1 gap
                h_ns = wp.tile([128, 2 * B], B16, tag="h_ns", name="h_ns")
                for m in range(2):
                    nc.scalar.activation(h_ns[:, m * B:(m + 1) * B],
                                         psC[:, m * B:(m + 1) * B], AF.Tanh)
                # NS half first: it heads the (binding) ys cycle; the mean
                # side has slack and absorbs NM landing later
                i_ns0 = mm(psE[:, B:], w["ns2_k0"][:, 128:], h_ns[:, 0:B],
                           start=True, stop=False)
                mm(psE[:, B:], w["ns2_k1"][:, 128:], h_ns[:, B:],
                   start=False, stop=False)
                i_ne = mm(psE[:, 0:B], w["neg_eye"][:], yode[:],
                          start=False, stop=False)
                add_dep_helper(i_ne.ins, i_ns0.ins, False, "bank-start")
                mm(psE[:, 0:B], w["ns2_k0"][:, 0:128], h_ns[:, 0:B],
                   start=False, stop=False)
                mm(psE[:, 0:B], w["ns2_k1"][:, 0:128], h_ns[:, B:],
                   start=False, stop=True)

                # mean channel: Ym' = Yode + G*(NM + bm - Yode); psE-NM
                # already holds NM - Yode via neg_eye, so ONE stt + adds
                pm = wp.tile([L, B], B16, tag="pm", name="pm")
                nc.vector.scalar_tensor_tensor(
                    pm[:], psE[:, 0:B], w["ns2_bm"][:, 0:1], g[:],
                    op0=OP.add, op1=OP.mult)
                nc.vector.tensor_tensor(ym[nxt][:], yode[:], pm[:], op=OP.add)
                if not last:
                    for m in range(2):
                        ms = slice(m * 128, (m + 1) * 128)
                        mm(psBn[:, m * B:(m + 1) * B], w["ode1_w"][:, ms], pm[:],
                           start=False, stop=(m == 1))
                    psB = psBn

                # std channel: sp(z)+1e-6 = relu(z) + ln2*u + (1+u)*2^{-u}
                # + (1e-6 - 1),  u = e^{-|z|}  (exact one-Newton log1p(e^z)).
                # Ys' = (P0 + T1) + Q: P0 = Ys + G*(rl + c - Ys) via Pool,
                # T1 = (ln2*G)*u, Q = (G*(1+u))*v, v = 2^{-u}; only Q and
                # two adds trail the exps.
                # |z| on ACT (Abs is in the exp table set): abs -> u -> v are
                # three back-to-back ACT ops, no cross-engine hops between.
                zb = wp.tile([L, B], F32, tag="zb", name="zb")
                nc.scalar.activation(zb[:], psE[:, B:], AF.Abs,
                                     bias=w["ns2_bs"][:, 0:1])
                u_e = wp.tile([L, B], B16, tag="u_e", name="u_e")
                nc.scalar.activation(u_e[:], zb[:], AF.Exp, scale=-1.0)
                v_e = wp.tile([L, B], B16, tag="v_e", name="v_e")
                nc.scalar.activation(v_e[:], u_e[:], AF.Exp, scale=-LN2)
                # DVE P0 path in parallel with the ACT chain:
                # Ppre = ys + g*(rl + c - ys), ready before v lands
                rl = wp.tile([L, B], F32, tag="rl", name="rl")
                nc.vector.tensor_scalar(rl[:], psE[:, B:], w["ns2_bs"][:, 0:1],
                                        0.0, op0=OP.add, op1=OP.max)
                sB = wp.tile([L, B], F32, tag="sB", name="sB")
                nc.vector.scalar_tensor_tensor(sB[:], rl[:], CC, ys[cur][:],
                                               op0=OP.add, op1=OP.subtract)
                # slack-tolerant fp32 products on the otherwise idle Pool
                P0a = wp.tile([L, B], F32, tag="P0a", name="P0a")
                nc.gpsimd.tensor_tensor(P0a[:], sB[:], g[:], op=OP.mult)
                Ppre = wp.tile([L, B], F32, tag="Ppre", name="Ppre")
                nc.gpsimd.tensor_tensor(Ppre[:], ys[cur][:], P0a[:], op=OP.add)
                # after u (during v's ACT): T1 = (ln2*g)*u, gw = (1+u)*g;
                # after v only Q = gw*v -- A2 and the final add are deferred
                # into the next step's emission (see above)
                T1 = wp.tile([L, B], B16, tag="T1", name="T1")
                nc.vector.tensor_tensor(T1[:], gl[:], u_e[:], op=OP.mult)
                gw = wp.tile([L, B], B16, tag="gw", name="gw")
                nc.vector.scalar_tensor_tensor(gw[:], u_e[:], 1.0, g[:],
                                               op0=OP.add, op1=OP.mult)
                Q_ = wp.tile([L, B], B16, tag="Q_", name="Q_")
                prev_iQ = nc.vector.tensor_tensor(Q_[:], gw[:], v_e[:], op=OP.mult)
                prev_tail = (Ppre, T1, Q_)
                prev_rg2k1 = i_rg2k1

            # ---- final transform ----
            fin = n_tp % 2
            if prev_tail is not None:
                pPpre, pT1, pQ = prev_tail
                A2 = wp.tile([L, B], F32, tag="A2", name="A2fin")
                nc.vector.tensor_tensor(A2[:], pPpre[:], pT1[:], op=OP.add)
                nc.vector.tensor_tensor(ys[fin][:], A2[:], pQ[:], op=OP.add)
            psB = pp.tile([128, 2 * B], F32, tag="psB", name="psB")
            for m in range(2):
                sl = psB[:, m * B:(m + 1) * B]
                ms = slice(m * 128, (m + 1) * 128)
                mm(sl, w["tz1_b"][:, ms], ones_row[:], start=True, stop=False)
                mm(sl, w["tz1_k0"][:, ms], ym[fin][:], start=False, stop=False)
                mm(sl, w["tz1_k1"][:, ms], ys[fin][:], start=False, stop=True)
            h_tz = wp.tile([128, 2 * B], B16, tag="h_ode", name="h_tz")
            nc.scalar.activation(h_tz[:], psB[:], AF.Tanh)
            psE = pp.tile([128, 2 * B], F32, tag="psE", name="psE2")
            for m in range(2):
                sl = psE[:, m * B:(m + 1) * B]
                ms = slice(m * 128, (m + 1) * 128)
                mm(sl, w["tz2_k0"][:, ms], h_tz[:, 0:B], start=True, stop=False)
                mm(sl, w["tz2_k1"][:, ms], h_tz[:, B:], start=False, stop=True)
            o_m = wp.tile([L, B], F32, tag="o_m", name="o_m")
            nc.scalar.activation(o_m[:], psE[:, 0:B], AF.Identity,
                                 bias=w["tz2_bm"][:, 0:1])
            o_s = wp.tile([L, B], F32, tag="o_s", name="o_s")
            nc.scalar.activation(o_s[:], psE[:, B:], AF.Abs,
                                 bias=w["tz2_bs"][:, 0:1])
            nc.sync.dma_start(d_om[:], o_m[:])
            nc.sync.dma_start(d_os[:], o_s[:])

    nc.compile()
    return nc


# --------------------------------------------------------------------------
# host-side packing
# --------------------------------------------------------------------------
def _dts(obs, n_tp):
    F = np.float32
    dd = (obs[:-1] - obs[1:])[::-1]
    return np.concatenate([np.full((1,), -0.01, F), dd]).astype(F)


def _prep_in_maps(inputs, n_tp):
    F = np.float32
    d = {k: np.ascontiguousarray(np.asarray(v, F)) for k, v in inputs.items()}
    data = d["data"][:, :n_tp]

    # [t, c, subj], reversed in time, bf16
    x_rev = np.ascontiguousarray(data.transpose(1, 2, 0)[::-1]).astype(BF)
    # observation mask per (t, subj), broadcast to 128 partitions
    m_t = (data[:, :, HALF:].sum(axis=2) > 0).astype(BF)  # [subj, t]
    m_rev = m_t.T[::-1]                                   # [t, subj]
    m_bc = np.ascontiguousarray(
        np.broadcast_to(m_rev[:, None, :], (n_tp, 128, N_SUBJ)))

    def kx(w1, b1):
        return np.vstack([w1[2 * L:], b1[None, :]])

    bf = {
        "ug1_k0": d["ug_w1"][:L], "ug1_k1": d["ug_w1"][L:2 * L],
        "ug1_kx": kx(d["ug_w1"], d["ug_b1"]),
        "rg1_k0": d["rg_w1"][:L], "rg1_k1": d["rg_w1"][L:2 * L],
        "rg1_kx": kx(d["rg_w1"], d["rg_b1"]),
        "ns1_k0": d["ns_w1"][:L] * F(0.5), "ns1_k1": d["ns_w1"][L:2 * L] * F(0.5),
        "ns1_kx": kx(d["ns_w1"], d["ns_b1"]),
        "ode1_w": d["ode_w1"], "ode_b2r": d["ode_b2"][None, :],
        "ode1_b1r0": d["ode_b1"][None, :128], "ode1_b1r1": d["ode_b1"][None, 128:],
        "neg_eye": -np.eye(L, dtype=F),
        "ode2_k0": d["ode_w2"][:128], "ode2_k1": d["ode_w2"][128:],
        "ug2_k0": d["ug_w2"][:128], "ug2_k1": d["ug_w2"][128:],
        "rg2_k0": d["rg_w2"][:128], "rg2_k1": d["rg_w2"][128:],
        "ns2_k0": d["ns_w2"][:128], "ns2_k1": d["ns_w2"][128:],
        "tz1_k0": d["tz_w1"][:L], "tz1_k1": d["tz_w1"][L:],
        "tz1_b": d["tz_b1"][None, :],
        "tz2_k0": d["tz_w2"][:128], "tz2_k1": d["tz_w2"][128:],
    }
    shared = {k: np.ascontiguousarray(v.astype(BF)) for k, v in bf.items()}
    shared["ug2_bc"] = np.ascontiguousarray(d["ug_b2"][:, None] * F(0.5))
    shared["rg2_bc"] = np.ascontiguousarray(d["rg_b2"][:, None] * F(0.5))
    shared["ns2_bm"] = np.ascontiguousarray(d["ns_b2"][:L, None])
    shared["ns2_bs"] = np.ascontiguousarray(d["ns_b2"][L:, None])
    shared["tz2_bm"] = np.ascontiguousarray(d["tz_b2"][:L, None])
    shared["tz2_bs"] = np.ascontiguousarray(d["tz_b2"][L:, None])

    in_maps = []
    for c in range(N_CORES):
        m = dict(shared)
        m["x_rev"] = np.ascontiguousarray(x_rev[:, :, c * B:(c + 1) * B])
        m["m_bc"] = np.ascontiguousarray(m_bc[:, :, c * B:(c + 1) * B])
        in_maps.append(m)
    return in_maps


def kernel(**inputs):
    from concourse.bass_utils import run_bass_kernel_spmd

    obs = np.asarray(inputs["obs_tps"], np.float32)[:N_TP]
    dts = _dts(obs, N_TP)
    key = (N_TP, tuple(np.asarray(dts, np.float64).tolist()))
    if key not in _CACHE:
        _CACHE[key] = _build(N_TP, dts)
    nc = _CACHE[key]

    in_maps = _prep_in_maps(inputs, N_TP)
    res = run_bass_kernel_spmd(nc, in_maps, list(range(N_CORES)))
    outs = res.results

    mean = np.empty((1, N_SUBJ, L), np.float32)
    std = np.empty((1, N_SUBJ, L), np.float32)
    for c in range(N_CORES):
        mean[0, c * B:(c + 1) * B] = outs[c]["out_m"].T
        std[0, c * B:(c + 1) * B] = outs[c]["out_s"].T
    return mean, std


# revision 46
# speedup vs baseline: 1.0619x; 1.0619x over previous
"""Trainium2 Bass kernel for an ODE-RNN encoder (z0 posterior).

Model: 128-step reversed-time GRU-like recurrence with an Euler ODE step on
the mean channel, then a final transform producing (mean_z0, std_z0).

Strategy: data-parallel over the subject (batch) dim across 8 NeuronCores,
weights replicated.  Everything runs on-chip in a transposed layout
([feature, batch], batch=256 on the free dim).  The step is latency-bound
by the recurrence cycle (ys' -> gate-1 k1 matmuls -> h_r -> rg2 -> tur_r
-> reset products -> ns1 -> h_ns -> ns2 -> softplus -> ys'), so every
choice below shortens that cycle or moves work off it (1.79ms -> 1.49ms):
- the ode1 pre-act psB is accumulated in the DISTRIBUTED form
  ode1@Ym' == ode1@Yode + ode1@pm (+ b1 via K=1 row matmuls), so step
  t+1's hidden pre-act is mostly built mid-step-t and only the small
  ode1@pm part trails the blend; Yode is ONE DVE op (psF carries b2 via
  a K=1 row matmul);
- softplus(x) = log1p(e^x) via one Newton step needs only TWO exps:
  (1+e^x)*e^{-y0} == (1+u)*2^{-u} with u = e^{-|x|} holds exactly for
  both signs (y0 = relu(x) + ln2*u).  ACT computes |z| (Abs), u, v
  back-to-back; the DVE tail is in P/Q form with the final two adds
  DEFERRED into the next step's emission so Yode never queues behind
  the tail in the DVE's in-order stream;
- reset products via host-side 0.5-prescale of ns1_k0/k1:
  r*state == 0.5*(1+tanh)*state becomes one scalar_tensor_tensor each;
- the update-gate factor G = mask * (0.5 - 0.5*tanh(z/2)) (== 1-sigmoid);
  the observation mask is host-precomputed and DMA'd as a [128,B] tile;
- mean blend: psE-NM accumulates NM - Yode via a neg-identity matmul, so
  Ym' = Yode + (psE+bm)*G is two DVE ops;
- m-half wavefront on the r-gate and ns stages (tanh per 128-row half,
  layer-2 matmuls start after the first half); add_dep_helper pins the
  scheduler where its cost model mis-orders (h_u after h_r-m1, next
  step's x-matmuls after rg2-k1, PSUM bank-start ordering);
- slack-tolerant softplus products run on the otherwise idle Pool engine
  (plain fp32 tensor_tensor only -- other ops are ISA-invalid there);
- matmul operands and the recurrent state are bf16 (fp32 PSUM
  accumulate): fp32 matmuls lower to TWO half-speed PE passes, bf16 is
  single-pass with fast weight load.
"""
import sys
import numpy as np
import ml_dtypes

for _p in ("/opt/trn_rl_repo", "/root/.axon_site/_ro/trn_rl_repo"):
    if _p not in sys.path:
        sys.path.append(_p)

N_SUBJ, N_TP, INPUT_DIM, LATENT, N_UNIT = 2048, 128, 64, 128, 256
HALF = INPUT_DIM // 2
N_CORES = 8
B = N_SUBJ // N_CORES          # 256 subjects per core (free dim)
L = LATENT
LN2 = float(np.log(2.0))
BF = ml_dtypes.bfloat16

_CACHE = {}


# --------------------------------------------------------------------------
# Bass program
# --------------------------------------------------------------------------
def _build(n_tp, dts):
    import concourse.mybir as mybir
    from concourse import bacc, tile

    F32 = mybir.dt.float32
    B16 = mybir.dt.bfloat16
    AF = mybir.ActivationFunctionType
    OP = mybir.AluOpType

    # Bacc (not plain Bass): its compile() legalizes the TRN2 one-sync-wait-
    # per-instruction limit (event-semaphore splitting, matmul-wait moves).
    nc = bacc.Bacc(None)

    # ---- DRAM I/O ----
    d_x = nc.dram_tensor("x_rev", [n_tp, INPUT_DIM, B], B16, kind="ExternalInput")
    d_m = nc.dram_tensor("m_bc", [n_tp, 128, B], B16, kind="ExternalInput")

    bspec = {  # bf16 weights (matmul operands)
        "ug1_k0": [L, N_UNIT], "ug1_k1": [L, N_UNIT], "ug1_kx": [INPUT_DIM + 1, N_UNIT],
        "rg1_k0": [L, N_UNIT], "rg1_k1": [L, N_UNIT], "rg1_kx": [INPUT_DIM + 1, N_UNIT],
        "ns1_k0": [L, N_UNIT], "ns1_k1": [L, N_UNIT], "ns1_kx": [INPUT_DIM + 1, N_UNIT],
        "ode1_w": [L, N_UNIT], "ode_b2r": [1, 128],
        "ode1_b1r0": [1, 128], "ode1_b1r1": [1, 128],
        "neg_eye": [L, L],
        "ode2_k0": [128, L], "ode2_k1": [128, L],
        "ug2_k0": [128, L], "ug2_k1": [128, L],
        "rg2_k0": [128, L], "rg2_k1": [128, L],
        "ns2_k0": [128, 2 * L], "ns2_k1": [128, 2 * L],
        "tz1_k0": [L, N_UNIT], "tz1_k1": [L, N_UNIT], "tz1_b": [1, N_UNIT],
        "tz2_k0": [128, 2 * L], "tz2_k1": [128, 2 * L],
    }
    fspec = {  # fp32 per-partition columns (ACT bias / DVE scalar APs)
        "ug2_bc": [128, 1], "rg2_bc": [128, 1],
        "ns2_bm": [128, 1], "ns2_bs": [128, 1], "tz2_bm": [128, 1], "tz2_bs": [128, 1],
    }
    d_w = {k: nc.dram_tensor(k, v, B16, kind="ExternalInput") for k, v in bspec.items()}
    d_w.update({k: nc.dram_tensor(k, v, F32, kind="ExternalInput")
                for k, v in fspec.items()})

    d_om = nc.dram_tensor("out_m", [L, B], F32, kind="ExternalOutput")
    d_os = nc.dram_tensor("out_s", [L, B], F32, kind="ExternalOutput")

    CC = float(np.float32(1e-6) - np.float32(1.0))

    with tile.TileContext(nc) as tc:
        with (
            tc.tile_pool(name="const", bufs=1) as cp,
            tc.tile_pool(name="work", bufs=3) as wp,
            tc.tile_pool(name="ps", bufs=1, space="PSUM") as pp,
        ):
            # ---- resident constants / weights ----
            w = {}
            for k, shp in bspec.items():
                w[k] = cp.tile(shp, B16, tag=k, name=k)
                nc.sync.dma_start(w[k][:], d_w[k][:])
            for k, shp in fspec.items():
                w[k] = cp.tile(shp, F32, tag=k, name=k)
                nc.sync.dma_start(w[k][:], d_w[k][:])
            ones_row = cp.tile([1, B], B16, tag="ones_row", name="ones_row")
            nc.vector.memset(ones_row[:], 1.0)

            xbufs = []
            for j in range(3):
                xb = cp.tile([INPUT_DIM + 1, B], B16, tag=f"xb{j}", name=f"xb{j}")
                nc.vector.memset(xb[INPUT_DIM:, :], 1.0)
                xbufs.append(xb)
            mbufs = [cp.tile([128, B], B16, tag=f"mb{j}", name=f"mb{j}")
                     for j in range(3)]

            # state lives in bf16 (the bf16 matmul-input rounding dominates;
            # bf16 state adds nothing measurable)
            ym = [cp.tile([L, B], B16, tag=f"ym{i}", name=f"ym{i}") for i in range(2)]
            ys = [cp.tile([L, B], B16, tag=f"ys{i}", name=f"ys{i}") for i in range(2)]
            nc.vector.memset(ym[0][:], 0.0)
            nc.vector.memset(ys[0][:], 0.0)

            mm = nc.tensor.matmul

            # Warm the PE's clock past every weight DMA with K=1 dummy
            # matmuls so steady-state matmuls only wait on one producer.
            scr = pp.tile([1, 16], F32, tag="scr", name="scr")
            for k in bspec:
                mm(scr[0:1, 0:1], w[k][0:1, 0:1], w[k][0:1, 1:2],
                   start=True, stop=True)
            # DVE/ACT read fp32 DMA-produced columns: warm those clocks too
            nf = len(fspec)
            warm_dv = cp.tile([1, 2 * nf], F32, tag="warm_dv", name="warm_dv")
            for j, k in enumerate(fspec):
                nc.vector.tensor_copy(warm_dv[0:1, j:j + 1], w[k][0:1, 0:1])
                nc.scalar.copy(warm_dv[0:1, nf + j:nf + j + 1], w[k][0:1, 0:1])

            # ---- the recurrence ----
            # psB (ode1 hidden pre-act) is accumulated via the distributed
            # form ode1@Ym' == ode1@Yode + ode1@pm (+ b1 K=1 rows), so step
            # t+1's psB starts filling mid-step-t and only the ode1@pm part
            # sits on the recurrence cycle.  Emission order == scheduler
            # priority: within each engine, ops are emitted in intended
            # steady-state execution order.
            from concourse.tile_rust import add_dep_helper
            prev_rg2k1 = None
            prev_tail = None
            psB = pp.tile([128, 2 * B], F32, tag="psB", name="psB_init")
            for m in range(2):
                mm(psB[:, m * B:(m + 1) * B], w[f"ode1_b1r{m}"][:],
                   ones_row[:], start=(m == 0), stop=(m == 1))
            for t in range(n_tp):
                cur, nxt = t % 2, (t + 1) % 2
                last = t == n_tp - 1
                dt_t = float(dts[t])
                xb = xbufs[t % 3]
                mb = mbufs[t % 3]
                nc.sync.dma_start(xb[:INPUT_DIM, :], d_x[t])
                nc.sync.dma_start(mb[:], d_m[t])
                # absorb the x-DMA wait into a K=1 dummy
                mm(scr[0:1, 0:1], xb[0:1, 0:1], xb[0:1, 1:2], start=True, stop=True)

                # PSUM banks (one start=True per bank per step)
                psAr = pp.tile([128, 2 * B], F32, tag="psAr", name="psAr")
                psAu = pp.tile([128, 2 * B], F32, tag="psAu", name="psAu")
                psC = pp.tile([128, 2 * B], F32, tag="psC", name="psC")
                psD = pp.tile([128, 2 * B], F32, tag="psD", name="psD")
                psE = pp.tile([128, 2 * B], F32, tag="psE", name="psE")
                psF = pp.tile([128, B], F32, tag="psF", name="psF")

                # ODE hidden of THIS step: psB was fully accumulated during
                # step t-1; single merged tanh (b1 already in psB)
                h_ode = wp.tile([128, 2 * B], B16, tag="h_ode", name="h_ode")
                nc.scalar.activation(h_ode[:], psB[:], AF.Tanh)

                # psF: b2 row first (ready instantly, opens the bank), then
                # ode2 halves as h_ode lands
                i_b2 = mm(psF[:], w["ode_b2r"][:], ones_row[:],
                          start=True, stop=False)
                # x-parts of the u/r gates: ready early (banks freed by the
                # previous step's h_r/h_u tanhs), keep the PE fed -- but not
                # at the cost of delaying the previous step's rg2 tail
                for net, ps in (("rg1", psAr), ("ug1", psAu)):
                    for m in range(2):
                        ms = slice(m * 128, (m + 1) * 128)
                        i_kx = mm(ps[:, m * B:(m + 1) * B], w[net + "_kx"][:, ms],
                                  xb[:], start=(m == 0), stop=False)
                        if net == "rg1" and m == 0 and prev_rg2k1 is not None:
                            add_dep_helper(i_kx.ins, prev_rg2k1.ins, False,
                                           "pe-order")
                for m in range(2):
                    mm(psF[:], w[f"ode2_k{m}"][:], h_ode[:, m * B:(m + 1) * B],
                       start=False, stop=(m == 1))

                # Yode = Ym + dt*(ode_out + b2): ONE DVE op
                yode = wp.tile([L, B], B16, tag="yode", name="yode")
                nc.vector.scalar_tensor_tensor(yode[:], psF[:], dt_t, ym[cur][:],
                                               op0=OP.mult, op1=OP.add)
                # deferred ys' of the previous step: emitted HERE so it sits
                # after yode in the DVE static order (yode must not queue
                # behind the whole previous tail)
                if prev_tail is not None:
                    pPpre, pT1, pQ = prev_tail
                    A2 = wp.tile([L, B], F32, tag="A2", name="A2")
                    nc.vector.tensor_tensor(A2[:], pPpre[:], pT1[:], op=OP.add)
                    nc.vector.tensor_tensor(ys[cur][:], A2[:], pQ[:], op=OP.add)
                    prev_tail = None

                # gate layer 1 state parts: yode (k0) BEFORE ys (k1) in PE
                # order -- yode is ready first in steady state
                for net, ps in (("rg1", psAr), ("ug1", psAu)):
                    for m in range(2):
                        ms = slice(m * 128, (m + 1) * 128)
                        mm(ps[:, m * B:(m + 1) * B], w[net + "_k0"][:, ms], yode[:],
                           start=False, stop=False)
                # next step's psB state part: ode1 @ Yode + b1 rows
                if not last:
                    psBn = pp.tile([128, 2 * B], F32, tag="psB", name=f"psBn{t}")
                    i_y0 = None
                    for m in range(2):
                        ms = slice(m * 128, (m + 1) * 128)
                        i_y = mm(psBn[:, m * B:(m + 1) * B], w["ode1_w"][:, ms],
                                 yode[:], start=(m == 0), stop=False)
                        if m == 0:
                            i_y0 = i_y
                    for m in range(2):
                        i_b1 = mm(psBn[:, m * B:(m + 1) * B],
                                  w[f"ode1_b1r{m}"][:], ones_row[:],
                                  start=False, stop=False)
                        # bank-start order: the b1 rows are ready from t=0
                        # and must not be scheduled before the start=True mm
                        add_dep_helper(i_b1.ins, i_y0.ins, False, "bank-start")
                for net, ps in (("rg1", psAr), ("ug1", psAu)):
                    for m in range(2):
                        ms = slice(m * 128, (m + 1) * 128)
                        mm(ps[:, m * B:(m + 1) * B], w[net + "_k1"][:, ms], ys[cur][:],
                           start=False, stop=(m == 1))

                # r-gate layer 2 with m-half wavefront; h_u after h_r-m1
                h_r = wp.tile([128, 2 * B], B16, tag="h_r", name="h_r")
                t_ur = wp.tile([128, 2 * B], B16, tag="t_ur", name="t_ur")
                i_hr1 = i_rg2k1 = None
                for m in range(2):
                    i_hr1 = nc.scalar.activation(h_r[:, m * B:(m + 1) * B],
                                                 psAr[:, m * B:(m + 1) * B], AF.Tanh)
                    i_rg2k1 = mm(psD[:, B:], w[f"rg2_k{m}"][:],
                                 h_r[:, m * B:(m + 1) * B],
                                 start=(m == 0), stop=(m == 1))
                nc.scalar.activation(t_ur[:, B:], psD[:, B:], AF.Tanh,
                                     bias=w["rg2_bc"][:, 0:1], scale=0.5)
                h_u = wp.tile([128, 2 * B], B16, tag="h_u", name="h_u")
                i_hu = nc.scalar.activation(h_u[:], psAu[:], AF.Tanh)
                # ACT order: the off-cycle merged h_u must not run between
                # the two h_r halves (the r path is the critical cycle)
                add_dep_helper(i_hu.ins, i_hr1.ins, False, "act-order")
                for m in range(2):
                    mm(psD[:, 0:B], w[f"ug2_k{m}"][:], h_u[:, m * B:(m + 1) * B],
                       start=False, stop=(m == 1))
                nc.scalar.activation(t_ur[:, 0:B], psD[:, 0:B], AF.Tanh,
                                     bias=w["ug2_bc"][:, 0:1], scale=0.5)

                # reset products via the prescale trick: ns1_k0/k1 carry a
                # host-side 0.5 factor, so r.state == 0.5*(1+tanh)*state
                # becomes ONE stt per channel: (tanh + 1) * state
                am = wp.tile([L, B], B16, tag="am", name="am")
                nc.vector.scalar_tensor_tensor(am[:], t_ur[:, B:], 1.0, yode[:],
                                               op0=OP.add, op1=OP.mult)
                a_s = wp.tile([L, B], B16, tag="a_s", name="a_s")
                nc.vector.scalar_tensor_tensor(a_s[:], t_ur[:, B:], 1.0, ys[cur][:],
                                               op0=OP.add, op1=OP.mult)
                # G = m * (0.5 - 0.5*t_u)   (== m * (1 - sigmoid(zU)))
                q_u = wp.tile([L, B], B16, tag="q_u", name="q_u")
                nc.vector.tensor_scalar(q_u[:], t_ur[:, 0:B], -0.5, 0.5,
                                        op0=OP.mult, op1=OP.add)
                g = wp.tile([L, B], B16, tag="g", name="g")
                nc.vector.tensor_tensor(g[:], q_u[:], mb[:L, :], op=OP.mult)
                gl = wp.tile([L, B], B16, tag="gl", name="gl")
                nc.vector.tensor_scalar(gl[:], g[:], LN2, None, op0=OP.mult)

                # ns1: x-part (psC WAR on previous h_ns clears mid-prev-step)
                # then the r-gated state parts, am before as per m-half
                for m in range(2):
                    ms = slice(m * 128, (m + 1) * 128)
                    mm(psC[:, m * B:(m + 1) * B], w["ns1_kx"][:, ms], xb[:],
                       start=(m == 0), stop=False)
                for m in range(2):
                    sl = psC[:, m * B:(m + 1) * B]
                    ms = slice(m * 128, (m + 1) * 128)
                    mm(sl, w["ns1_k0"][:, ms], am[:], start=False, stop=False)
                    mm(sl, w["ns1_k1"][:, ms], a_s[:], start=False, stop=(m == 1))

                # ns layer 2 with m-half wavefront; NM first (mean cycle),
                # neg_eye (-Yode, ready early) fills the h_ns-m1 gap
                h_ns = wp.tile([128, 2 * B], B16, tag="h_ns", name="h_ns")
                for m in range(2):
                    nc.scalar.activation(h_ns[:, m * B:(m + 1) * B],
                                         psC[:, m * B:(m + 1) * B], AF.Tanh)
                i_nm0 = mm(psE[:, 0:B], w["ns2_k0"][:, 0:128], h_ns[:, 0:B],
                           start=True, stop=False)
                i_ne = mm(psE[:, 0:B], w["neg_eye"][:], yode[:],
                          start=False, stop=False)
                add_dep_helper(i_ne.ins, i_nm0.ins, False, "bank-start")
                mm(psE[:, 0:B], w["ns2_k1"][:, 0:128], h_ns[:, B:],
                   start=False, stop=False)
                mm(psE[:, B:], w["ns2_k0"][:, 128:], h_ns[:, 0:B],
                   start=False, stop=False)
                mm(psE[:, B:], w["ns2_k1"][:, 128:], h_ns[:, B:],
                   start=False, stop=(m == 1))

                # mean channel: Ym' = Yode + G*(NM + bm - Yode); psE-NM
                # already holds NM - Yode via neg_eye, so ONE stt + adds
                pm = wp.tile([L, B], B16, tag="pm", name="pm")
                nc.vector.scalar_tensor_tensor(
                    pm[:], psE[:, 0:B], w["ns2_bm"][:, 0:1], g[:],
                    op0=OP.add, op1=OP.mult)
                nc.vector.tensor_tensor(ym[nxt][:], yode[:], pm[:], op=OP.add)
                if not last:
                    for m in range(2):
                        ms = slice(m * 128, (m + 1) * 128)
                        mm(psBn[:, m * B:(m + 1) * B], w["ode1_w"][:, ms], pm[:],
                           start=False, stop=(m == 1))
                    psB = psBn

                # std channel: sp(z)+1e-6 = relu(z) + ln2*u + (1+u)*2^{-u}
                # + (1e-6 - 1),  u = e^{-|z|}  (exact one-Newton log1p(e^z)).
                # Ys' = (P0 + T1) + Q: P0 = Ys + G*(rl + c - Ys) via Pool,
                # T1 = (ln2*G)*u, Q = (G*(1+u))*v, v = 2^{-u}; only Q and
                # two adds trail the exps.
                # |z| on ACT (Abs is in the exp table set): abs -> u -> v are
                # three back-to-back ACT ops, no cross-engine hops between.
                zb = wp.tile([L, B], F32, tag="zb", name="zb")
                nc.scalar.activation(zb[:], psE[:, B:], AF.Abs,
                                     bias=w["ns2_bs"][:, 0:1])
                u_e = wp.tile([L, B], B16, tag="u_e", name="u_e")
                nc.scalar.activation(u_e[:], zb[:], AF.Exp, scale=-1.0)
                v_e = wp.tile([L, B], B16, tag="v_e", name="v_e")
                nc.scalar.activation(v_e[:], u_e[:], AF.Exp, scale=-LN2)
                # DVE P0 path in parallel with the ACT chain:
                # Ppre = ys + g*(rl + c - ys), ready before v lands
                rl = wp.tile([L, B], F32, tag="rl", name="rl")
                nc.vector.tensor_scalar(rl[:], psE[:, B:], w["ns2_bs"][:, 0:1],
                                        0.0, op0=OP.add, op1=OP.max)
                sB = wp.tile([L, B], F32, tag="sB", name="sB")
                nc.vector.scalar_tensor_tensor(sB[:], rl[:], CC, ys[cur][:],
                                               op0=OP.add, op1=OP.subtract)
                # slack-tolerant fp32 products on the otherwise idle Pool
                P0a = wp.tile([L, B], F32, tag="P0a", name="P0a")
                nc.gpsimd.tensor_tensor(P0a[:], sB[:], g[:], op=OP.mult)
                Ppre = wp.tile([L, B], F32, tag="Ppre", name="Ppre")
                nc.gpsimd.tensor_tensor(Ppre[:], ys[cur][:], P0a[:], op=OP.add)
                # after u (during v's ACT): T1 = (ln2*g)*u, gw = (1+u)*g;
                # after v only Q = gw*v -- A2 and the final add are deferred
                # into the next step's emission (see above)
                T1 = wp.tile([L, B], B16, tag="T1", name="T1")
                nc.vector.tensor_tensor(T1[:], gl[:], u_e[:], op=OP.mult)
                gw = wp.tile([L, B], B16, tag="gw", name="gw")
                nc.vector.scalar_tensor_tensor(gw[:], u_e[:], 1.0, g[:],
                                               op0=OP.add, op1=OP.mult)
                Q_ = wp.tile([L, B], B16, tag="Q_", name="Q_")
                nc.vector.tensor_tensor(Q_[:], gw[:], v_e[:], op=OP.mult)
                prev_tail = (Ppre, T1, Q_)
                prev_rg2k1 = i_rg2k1

            # ---- final transform ----
            fin = n_tp % 2
            if prev_tail is not None:
                pPpre, pT1, pQ = prev_tail
                A2 = wp.tile([L, B], F32, tag="A2", name="A2fin")
                nc.vector.tensor_tensor(A2[:], pPpre[:], pT1[:], op=OP.add)
                nc.vector.tensor_tensor(ys[fin][:], A2[:], pQ[:], op=OP.add)
            psB = pp.tile([128, 2 * B], F32, tag="psB", name="psB")
            for m in range(2):
                sl = psB[:, m * B:(m + 1) * B]
                ms = slice(m * 128, (m + 1) * 128)
                mm(sl, w["tz1_b"][:, ms], ones_row[:], start=True, stop=False)
                mm(sl, w["tz1_k0"][:, ms], ym[fin][:], start=False, stop=False)
                mm(sl, w["tz1_k1"][:, ms], ys[fin][:], start=False, stop=True)
            h_tz = wp.tile([128, 2 * B], B16, tag="h_ode", name="h_tz")
            nc.scalar.activation(h_tz[:], psB[:], AF.Tanh)
            psE = pp.tile([128, 2 * B], F32, tag="psE", name="psE2")
            for m in range(2):
                sl = psE[:, m * B:(m + 1) * B]
                ms = slice(m * 128, (m + 1) * 128)
                mm(sl, w["tz2_k0"][:, ms], h_tz[:, 0:B], start=True, stop=False)
                mm(sl, w["tz2_k1"][:, ms], h_tz[:, B:], start=False, stop=True)
            o_m = wp.tile([L, B], F32, tag="o_m", name="o_m")
            nc.scalar.activation(o_m[:], psE[:, 0:B], AF.Identity,
                                 bias=w["tz2_bm"][:, 0:1])
            o_s = wp.tile([L, B], F32, tag="o_s", name="o_s")
            nc.scalar.activation(o_s[:], psE[:, B:], AF.Abs,
                                 bias=w["tz2_bs"][:, 0:1])
            nc.sync.dma_start(d_om[:], o_m[:])
            nc.sync.dma_start(d_os[:], o_s[:])

    nc.compile()
    return nc


# --------------------------------------------------------------------------
# host-side packing
# --------------------------------------------------------------------------
def _dts(obs, n_tp):
    F = np.float32
    dd = (obs[:-1] - obs[1:])[::-1]
    return np.concatenate([np.full((1,), -0.01, F), dd]).astype(F)


def _prep_in_maps(inputs, n_tp):
    F = np.float32
    d = {k: np.ascontiguousarray(np.asarray(v, F)) for k, v in inputs.items()}
    data = d["data"][:, :n_tp]

    # [t, c, subj], reversed in time, bf16
    x_rev = np.ascontiguousarray(data.transpose(1, 2, 0)[::-1]).astype(BF)
    # observation mask per (t, subj), broadcast to 128 partitions
    m_t = (data[:, :, HALF:].sum(axis=2) > 0).astype(BF)  # [subj, t]
    m_rev = m_t.T[::-1]                                   # [t, subj]
    m_bc = np.ascontiguousarray(
        np.broadcast_to(m_rev[:, None, :], (n_tp, 128, N_SUBJ)))

    def kx(w1, b1):
        return np.vstack([w1[2 * L:], b1[None, :]])

    bf = {
        "ug1_k0": d["ug_w1"][:L], "ug1_k1": d["ug_w1"][L:2 * L],
        "ug1_kx": kx(d["ug_w1"], d["ug_b1"]),
        "rg1_k0": d["rg_w1"][:L], "rg1_k1": d["rg_w1"][L:2 * L],
        "rg1_kx": kx(d["rg_w1"], d["rg_b1"]),
        "ns1_k0": d["ns_w1"][:L] * F(0.5), "ns1_k1": d["ns_w1"][L:2 * L] * F(0.5),
        "ns1_kx": kx(d["ns_w1"], d["ns_b1"]),
        "ode1_w": d["ode_w1"], "ode_b2r": d["ode_b2"][None, :],
        "ode1_b1r0": d["ode_b1"][None, :128], "ode1_b1r1": d["ode_b1"][None, 128:],
        "neg_eye": -np.eye(L, dtype=F),
        "ode2_k0": d["ode_w2"][:128], "ode2_k1": d["ode_w2"][128:],
        "ug2_k0": d["ug_w2"][:128], "ug2_k1": d["ug_w2"][128:],
        "rg2_k0": d["rg_w2"][:128], "rg2_k1": d["rg_w2"][128:],
        "ns2_k0": d["ns_w2"][:128], "ns2_k1": d["ns_w2"][128:],
        "tz1_k0": d["tz_w1"][:L], "tz1_k1": d["tz_w1"][L:],
        "tz1_b": d["tz_b1"][None, :],
        "tz2_k0": d["tz_w2"][:128], "tz2_k1": d["tz_w2"][128:],
    }
    shared = {k: np.ascontiguousarray(v.astype(BF)) for k, v in bf.items()}
    shared["ug2_bc"] = np.ascontiguousarray(d["ug_b2"][:, None] * F(0.5))
    shared["rg2_bc"] = np.ascontiguousarray(d["rg_b2"][:, None] * F(0.5))
    shared["ns2_bm"] = np.ascontiguousarray(d["ns_b2"][:L, None])
    shared["ns2_bs"] = np.ascontiguousarray(d["ns_b2"][L:, None])
    shared["tz2_bm"] = np.ascontiguousarray(d["tz_b2"][:L, None])
    shared["tz2_bs"] = np.ascontiguousarray(d["tz_b2"][L:, None])

    in_maps = []
    for c in range(N_CORES):
        m = dict(shared)
        m["x_rev"] = np.ascontiguousarray(x_rev[:, :, c * B:(c + 1) * B])
        m["m_bc"] = np.ascontiguousarray(m_bc[:, :, c * B:(c + 1) * B])
        in_maps.append(m)
    return in_maps


def kernel(**inputs):
    from concourse.bass_utils import run_bass_kernel_spmd

    obs = np.asarray(inputs["obs_tps"], np.float32)[:N_TP]
    dts = _dts(obs, N_TP)
    key = (N_TP, tuple(np.asarray(dts, np.float64).tolist()))
    if key not in _CACHE:
        _CACHE[key] = _build(N_TP, dts)
    nc = _CACHE[key]

    in_maps = _prep_in_maps(inputs, N_TP)
    res = run_bass_kernel_spmd(nc, in_maps, list(range(N_CORES)))
    outs = res.results

    mean = np.empty((1, N_SUBJ, L), np.float32)
    std = np.empty((1, N_SUBJ, L), np.float32)
    for c in range(N_CORES):
        mean[0, c * B:(c + 1) * B] = outs[c]["out_m"].T
        std[0, c * B:(c + 1) * B] = outs[c]["out_s"].T
    return mean, std
